# revision 3
# baseline (speedup 1.0000x reference)
"""ContrastiveLoss Trainium2 kernel.

Strategy (data-parallel over 8 NeuronCores):
  - 8 cores = 4 batches x 2 halves. Core c handles batch b=c//2, half h=c%2:
    2500 match pairs + 25000 non-match pairs.
  - Gather primitive: SWDGE vector-indirect DMA (`indirect_dma_start`). Each
    instruction carries a [128, k] int32 index tile and fetches one 64 B row
    (16 f32) per index: sample (p, j) lands at g[p, 16j:16j+16]. The SWDGE
    descriptor-generation cost is ~1 us fixed per instruction + 0.34 ns per
    row, so the kernel batches k=56 columns (7168 rows) per instruction
    instead of the naive one-column-per-instruction form (which was ~450 us
    of serial GPSIMD time).
  - A and B rows come from ONE concatenated DRAM tensor E = [outA[b]; outB[b]]
    (2N rows) so a single instruction gathers both sides of a chunk: columns
    0..27 hold A-block indices, columns 28..55 hold B-block indices + N.
  - Per-sample math on DVE/ACT (hidden under the gather stream):
      match partial  = sum((mA-mB)^2)              (DVE sub + masked ACT
        square with fused accumulate)
      nonmatch partial = sum(relu(0.5-||nA-nB||^2)) (DVE sub, ACT square,
        DVE grouped reduce over D=16, ACT relu with fused accumulate)
  - Padding: tail samples use index 0 on both sides; a host-built {0,1} mask
    zeroes pad match diffs exactly, and a host-built additive bias pushes pad
    nonmatch distances to 1e9 so the hinge is exactly 0.
  - Partition reduction via a ones-vector TensorE matmul into PSUM.
  - Each core outputs [1,2] raw partial sums; the host combines 8x2 scalars
    and applies the 1/5000 and 1/50000 normalizations.

kernel() takes the FULL (unsharded) inputs and returns the full output tuple
(contrastive_loss_sum, match_loss_sum, nonmatch_loss_sum) like the reference.
"""

import os

import numpy as np

# Problem constants (hardcoded per task spec).
B, N, D = 4, 307200, 16
M_MATCH, M_NONMATCH = 5000, 50000
MARGIN = 0.5
NON_MATCH_WEIGHT = 1.0
NCORES = 8

P = 128
MH = M_MATCH // 2  # 2500 match samples per core
NH = M_NONMATCH // 2  # 25000 nonmatch samples per core
MBLK = (MH + P - 1) // P  # 20 match blocks (last one partial: 2500=19*128+68)
NBLK = (NH + P - 1) // P  # 196 nonmatch blocks (25000=195*128+40)
M_REM = MH - (MBLK - 1) * P  # 68 real rows in last match block
N_REM = NH - (NBLK - 1) * P  # 40 real rows in last nonmatch block
CBLK = 28  # nonmatch blocks per compute chunk
NCH = NBLK // CBLK  # 7 chunks
assert NCH * CBLK == NBLK
# combined index tile: per chunk [A(28) | B(28)], then match [A(20) | B(20)]
TOTCOL = NCH * 2 * CBLK + 2 * MBLK  # 432
# SWDGE descriptor ring carveout: one nonmatch chunk generates 128*56=7168
# descriptors in a single instruction; size the ring to hold it.
DMA_SCRATCH = 131072

LAST_EXEC_NS = None

_CACHE = {}


def _build_nc():
    import concourse.bacc as bacc
    import concourse.mybir as mybir
    import concourse.tile as tile
    from concourse import bass

    f32 = mybir.dt.float32
    i32 = mybir.dt.int32
    X = mybir.AxisListType.X
    ADD = mybir.AluOpType.add
    MULT = mybir.AluOpType.mult
    Relu = mybir.ActivationFunctionType.Relu

    nc = bacc.Bacc(
        "TRN2",
        target_bir_lowering=False,
        debug=False,
        dynamic_dma_scratch_size=DMA_SCRATCH,
    )
    E = nc.dram_tensor("E", (2 * N, D), f32, kind="ExternalInput")
    idx = nc.dram_tensor("idx", (P, TOTCOL), i32, kind="ExternalInput")
    # pad handling: mmask is 1.0 for real match samples else 0.0;
    # npad adds 1e9 to pad nonmatch distances (hinge -> exactly 0)
    mmask = nc.dram_tensor("mmask", (P, MBLK), f32, kind="ExternalInput")
    npad = nc.dram_tensor("npad", (P, CBLK), f32, kind="ExternalInput")
    out = nc.dram_tensor("out", (1, 2), f32, kind="ExternalOutput")

    with tile.TileContext(nc) as tc:
        with (
            tc.tile_pool(name="idx", bufs=1) as idxp,
            tc.tile_pool(name="gath", bufs=3) as gp,
            tc.tile_pool(name="cmp", bufs=3) as cp,
            tc.tile_pool(name="sums", bufs=1) as sp,
            tc.tile_pool(name="psum", bufs=1, space="PSUM") as pp,
        ):
            # index tiles (HWDGE loads; keep Pool free for the gather stream)
            idx_t = idxp.tile([P, TOTCOL], i32)
            nc.sync.dma_start(idx_t[:], idx.ap())
            mmask_t = idxp.tile([P, MBLK], f32)
            nc.sync.dma_start(mmask_t[:], mmask.ap())
            npad_t = idxp.tile([P, CBLK], f32)
            nc.sync.dma_start(npad_t[:], npad.ap())

            sums = sp.tile([P, 1 + NCH], f32)
            margin_t = sp.tile([P, 1], f32)
            nc.vector.memset(margin_t[:], MARGIN)

            # --- nonmatch: one gather instruction per chunk of 28 blocks ---
            W = CBLK * D  # 448 columns per side
            for c in range(NCH):
                g = gp.tile([P, 2 * W], f32, tag="g")
                nc.gpsimd.indirect_dma_start(
                    out=g[:],
                    out_offset=None,
                    in_=E.ap(),
                    in_offset=bass.IndirectOffsetOnAxis(
                        ap=idx_t[:, c * 2 * CBLK : (c + 1) * 2 * CBLK], axis=0
                    ),
                )
                nd = cp.tile([P, W], f32, tag="nd")
                nc.vector.tensor_sub(nd[:], g[:, :W], g[:, W:])
                nsq = cp.tile([P, W], f32, tag="nsq")
                nc.scalar.square(nsq[:], nd[:])
                dist = cp.tile([P, CBLK], f32, tag="dist")
                nc.vector.tensor_reduce(
                    dist[:],
                    nsq[:].rearrange("p (s d) -> p s d", d=D),
                    axis=X,
                    op=ADD,
                )
                if c == NCH - 1:
                    # pad samples: add 1e9 to their distance so the hinge
                    # is exactly 0
                    nc.vector.tensor_add(dist[:], dist[:], npad_t[:])
                hng = cp.tile([P, CBLK], f32, tag="hng")
                nc.scalar.activation(
                    hng[:],
                    dist[:],
                    Relu,
                    bias=margin_t[:],
                    scale=-1.0,
                    accum_out=sums[:, 1 + c : 2 + c],
                )

            # --- match: 20 blocks, one gather instruction ---
            MW = MBLK * D  # 320 columns per side
            gm = gp.tile([P, 2 * MW], f32, tag="gm")
            nc.gpsimd.indirect_dma_start(
                out=gm[:],
                out_offset=None,
                in_=E.ap(),
                in_offset=bass.IndirectOffsetOnAxis(
                    ap=idx_t[:, NCH * 2 * CBLK : NCH * 2 * CBLK + 2 * MBLK], axis=0
                ),
            )
            md = cp.tile([P, MW], f32, tag="md")
            nc.vector.tensor_sub(md[:], gm[:, :MW], gm[:, MW:])
            # mask the pad samples exactly: mdm = md * mmask (broadcast over D)
            mdm = cp.tile([P, MW], f32, tag="mdm")
            nc.vector.tensor_tensor(
                out=mdm[:].rearrange("p (s d) -> p s d", d=D),
                in0=md[:].rearrange("p (s d) -> p s d", d=D),
                in1=mmask_t[:].unsqueeze(2).to_broadcast([P, MBLK, D]),
                op=MULT,
            )
            msq = cp.tile([P, MW], f32, tag="msq")
            nc.scalar.activation(
                msq[:],
                mdm[:],
                mybir.ActivationFunctionType.Square,
                accum_out=sums[:, 0:1],
            )

            # --- cross-partition reduction: ones[128,1].T @ sums[128,1+NCH] ---
            ones = sp.tile([P, 1], f32)
            nc.vector.memset(ones[:], 1.0)
            acc = pp.tile([1, 1 + NCH], f32, space="PSUM")
            nc.tensor.matmul(acc[:], lhsT=ones[:], rhs=sums[:], start=True, stop=True)
            res = sp.tile([1, 2], f32)
            nc.vector.tensor_copy(res[:, 0:1], acc[:, 0:1])
            nc.vector.tensor_reduce(res[:, 1:2], acc[:, 1 : 1 + NCH], axis=X, op=ADD)
            nc.sync.dma_start(out.ap(), res[:])

    nc.compile()
    return nc


def _get_nc():
    if "nc" not in _CACHE:
        _CACHE["nc"] = _build_nc()
    return _CACHE["nc"]


def _blocked(idx_1d, nblocks):
    """[n] -> [128, nblocks] with sample s at [s % 128, s // 128]; pad with 0."""
    out = np.zeros((P, nblocks), np.int32)
    n = idx_1d.shape[0]
    full = n // P
    out[:, :full] = idx_1d[: full * P].reshape(full, P).T
    rem = n - full * P
    if rem:
        out[:rem, full] = idx_1d[full * P :]
    return out


def _idx_map(mA, mB, nA, nB):
    """Build the combined [128, TOTCOL] index tile for one core.

    Per nonmatch chunk c: cols [c*56, c*56+28) are A-block indices for blocks
    c*28..c*28+27, cols [c*56+28, c*56+56) are the matching B-block indices
    shifted by N (B rows live at E[N:]). Match cols follow: [A(20) | B(20)+N].
    """
    bA = _blocked(nA, NBLK)
    bB = _blocked(nB, NBLK) + np.int32(N)
    out = np.empty((P, TOTCOL), np.int32)
    for c in range(NCH):
        out[:, c * 2 * CBLK : c * 2 * CBLK + CBLK] = bA[:, c * CBLK : (c + 1) * CBLK]
        out[:, c * 2 * CBLK + CBLK : (c + 1) * 2 * CBLK] = bB[
            :, c * CBLK : (c + 1) * CBLK
        ]
    base = NCH * 2 * CBLK
    out[:, base : base + MBLK] = _blocked(mA, MBLK)
    out[:, base + MBLK : base + 2 * MBLK] = _blocked(mB, MBLK) + np.int32(N)
    return out


def _in_maps(outA, outB, matchA, matchB, nonMatchA, nonMatchB):
    outA = np.asarray(outA, dtype=np.float32)
    outB = np.asarray(outB, dtype=np.float32)
    matchA = np.asarray(matchA).astype(np.int32)
    matchB = np.asarray(matchB).astype(np.int32)
    nonMatchA = np.asarray(nonMatchA).astype(np.int32)
    nonMatchB = np.asarray(nonMatchB).astype(np.int32)

    mmask = np.zeros((P, MBLK), np.float32)
    mmask[:, : MBLK - 1] = 1.0
    mmask[:M_REM, MBLK - 1] = 1.0
    npad = np.zeros((P, CBLK), np.float32)
    npad[N_REM:, CBLK - 1] = 1e9

    # E[b] = [outA[b]; outB[b]] — one gather source per batch, shared by the
    # two half-cores of that batch.
    Es = [
        np.ascontiguousarray(
            np.concatenate([outA[b], outB[b]], axis=0, dtype=np.float32)
        )
        for b in range(B)
    ]

    maps = []
    for c in range(NCORES):
        b, h = c // 2, c % 2
        maps.append(
            {
                "E": Es[b],
                "idx": _idx_map(
                    matchA[b, h * MH : (h + 1) * MH],
                    matchB[b, h * MH : (h + 1) * MH],
                    nonMatchA[b, h * NH : (h + 1) * NH],
                    nonMatchB[b, h * NH : (h + 1) * NH],
                ),
                "mmask": mmask,
                "npad": npad,
            }
        )
    return maps


def kernel(outA, outB, matchA, matchB, nonMatchA, nonMatchB):
    global LAST_EXEC_NS
    from concourse import bass_utils

    nc = _get_nc()
    maps = _in_maps(outA, outB, matchA, matchB, nonMatchA, nonMatchB)

    kwargs = {}
    if os.environ.get("KERNEL_TRACE", "0") == "1":
        kwargs["trace"] = True
    r = bass_utils.run_bass_kernel_spmd(
        nc, maps, core_ids=list(range(NCORES)), **kwargs
    )
    LAST_EXEC_NS = r.exec_time_ns

    partial = np.stack(
        [np.asarray(r.results[c]["out"]).ravel() for c in range(NCORES)]
    )
    match_loss = partial[:, 0].sum(dtype=np.float64) / M_MATCH
    nonmatch_loss = (
        NON_MATCH_WEIGHT * partial[:, 1].sum(dtype=np.float64) / M_NONMATCH
    )
    contrastive = match_loss + nonmatch_loss
    return (
        np.float32(contrastive),
        np.float32(match_loss),
        np.float32(nonmatch_loss),
    )
